# revision 2
# baseline (speedup 1.0000x reference)
"""Dense MLP forward (y = quantize(relu(x @ w + b))) on 8 TRN2 NeuronCores.

Strategy: pure data-parallel over the batch dim (1024 rows per core), w/b
replicated, no collectives. Host-side each core receives its x shard
*transposed* so the contraction dim lands on SBUF partitions with contiguous
DMA — zero on-chip transposes. Everything ships bf16 (inputs are fixed-point
with ~2e-2 harness tolerance; bf16 end-to-end measures ~2.9e-3 rel err),
halving HBM traffic vs f32 and removing the int16->f32r expansion dependency
of the previous version. Each core computes yT tiles:

  - matmuls in bf16, w chunks [128k,128n] stationary (FWL active), xT chunks
    [128k,512m] moving, accumulating over k into all 8 PSUM banks (8 n-groups
    in flight), k-major wave order in band 0 so the PE starts as soon as the
    first k-chunk DMA lands; band 1 uses a skewed schedule so group stops
    stagger and evictions overlap matmuls.
  - 3 junk warm-up matmuls fill the first-DMA window only (real data lands
    ~1.3us after the framework preamble; more warm-ups just delay real work).
  - epilogue per [128n, 512m] tile: relu(psum + b) in one op (bias is
    per-partition in the transposed layout), alternating ACT/DVE engines;
    the final two groups evict in halves on both engines in parallel with
    half-stores on both DMA queues so the post-last-matmul chain is short.

Host transposes each core's yT back and concatenates.
"""

import numpy as np
import ml_dtypes

import concourse.bacc as bacc
import concourse.tile as tile
from concourse import mybir
from concourse.bass_utils import run_bass_kernel_spmd

P = 128
B, D_IN, D_OUT = 8192, 1024, 1024
N_CORES = 8
M = B // N_CORES          # batch rows per core
KC = D_IN // P            # 8 k-chunks
NT = D_OUT // P           # 8 n-groups (PSUM partition tiles)
MB = 512                  # matmul moving free dim / PSUM bank width (fp32)
NUM_MB = M // MB          # 2 m-bands per core

N_WARMUP_MM = 3           # fills the first-input-DMA window only

F32 = mybir.dt.float32
BF16 = mybir.dt.bfloat16

_CACHE = {}


def build_bass():
    nc = bacc.Bacc("TRN2", target_bir_lowering=False, debug=False)

    xT_d = nc.dram_tensor("xT", [D_IN, M], BF16, kind="ExternalInput")
    w_d = nc.dram_tensor("w", [D_IN, D_OUT], BF16, kind="ExternalInput")
    b_d = nc.dram_tensor("b", [D_OUT], F32, kind="ExternalInput")
    yT_d = nc.dram_tensor("yT", [D_OUT, M], BF16, kind="ExternalOutput")

    with tile.TileContext(nc) as tc:
        with (
            tc.tile_pool(name="const", bufs=1) as cst,
            tc.tile_pool(name="wx", bufs=1) as wx,
            tc.tile_pool(name="outp", bufs=8) as outp,
            tc.tile_pool(name="ps", bufs=1, space="PSUM") as ps,
        ):
            # PE warm-up on junk data while the first input DMAs stream in
            zt = cst.tile([P, MB], BF16, tag="warm_src")
            nc.gpsimd.memset(zt, 0.0)
            warm_ps = ps.tile([P, MB], F32, tag="acc7")
            for _ in range(N_WARMUP_MM):
                nc.tensor.matmul(warm_ps, zt[:, :P], zt, start=True, stop=True)

            # bias: b[n] -> [p, c] with n = c*128 + p.
            # Issued on the ACT HWDGE ring so it doesn't delay w0 on SP.
            b_sb = cst.tile([P, NT], F32, tag="bias_raw")
            nc.scalar.dma_start(out=b_sb, in_=b_d.ap().rearrange("(c p) -> p c", p=P))

            # Inputs: one SBUF tile per k-chunk, w/x interleaved so early
            # waves unblock first. Full-width x chunks keep 2KB DMA lines.
            w_tiles = [wx.tile([P, D_OUT], BF16, tag=f"wc{c}", name=f"wc{c}") for c in range(KC)]
            x_tiles = [wx.tile([P, M], BF16, tag=f"xc{c}", name=f"xc{c}") for c in range(KC)]
            for c in range(KC):
                nc.sync.dma_start(out=w_tiles[c], in_=w_d.ap()[c * P : (c + 1) * P, :])
                nc.sync.dma_start(out=x_tiles[c], in_=xT_d.ap()[c * P : (c + 1) * P, :])

            def emit_mm(accs, mb, nt, c):
                nc.tensor.matmul(
                    accs[nt],
                    w_tiles[c][:, nt * P : (nt + 1) * P],
                    x_tiles[c][:, mb * MB : (mb + 1) * MB],
                    start=(c == 0),
                    stop=(c == KC - 1),
                )

            for mb in range(NUM_MB):
                accs = [ps.tile([P, MB], F32, tag=f"acc{nt}", name=f"acc{nt}") for nt in range(NT)]
                if mb == 0:
                    # k-major waves: 8 MMs per arriving chunk, one per n-group
                    for c in range(KC):
                        for nt in range(NT):
                            emit_mm(accs, mb, nt, c)
                else:
                    # skewed waves: group nt runs chunk c at wave t=nt+c, so
                    # stops stagger ~8 MMs apart and evictions overlap MMs
                    for t in range(KC + NT - 1):
                        for nt in range(NT):
                            c = t - nt
                            if 0 <= c < KC:
                                emit_mm(accs, mb, nt, c)
                for nt in range(NT):
                    # pipeline the tail groups' epilogues in half tiles so the
                    # last ACT/store chain after the final matmul is short
                    halves = 2 if (mb == NUM_MB - 1 and nt >= NT - 2) else 1
                    o = outp.tile([P, MB], BF16, tag="otile")
                    HW_ = MB // halves
                    for h in range(halves):
                        sl = slice(h * HW_, (h + 1) * HW_)
                        # relu(y + b); bias varies along partitions here.
                        # Alternate eviction engine (ACT / DVE) so PSUM banks
                        # release in parallel and the next band ramps sooner.
                        if nt % 2 == 0:
                            nc.scalar.activation(
                                o[:, sl],
                                accs[nt][:, sl],
                                mybir.ActivationFunctionType.Relu,
                                bias=b_sb[:, nt : nt + 1],
                                scale=1.0,
                            )
                        else:
                            nc.vector.tensor_scalar(
                                o[:, sl],
                                accs[nt][:, sl],
                                b_sb[:, nt : nt + 1],
                                0.0,
                                mybir.AluOpType.add,
                                mybir.AluOpType.max,
                            )
                        (nc.sync if (nt + h) % 2 == 0 else nc.scalar).dma_start(
                            out=yT_d.ap()[
                                nt * P : (nt + 1) * P,
                                mb * MB + h * HW_ : mb * MB + (h + 1) * HW_,
                            ],
                            in_=o[:, sl],
                        )

    nc.compile()
    return nc


def get_nc():
    if "nc" not in _CACHE:
        _CACHE["nc"] = build_bass()
    return _CACHE["nc"]


def make_in_maps(x, w, b):
    x = np.asarray(x, dtype=np.float32)
    w = np.asarray(w, dtype=np.float32)
    b = np.ascontiguousarray(b, dtype=np.float32)
    w_bf = np.ascontiguousarray(w.astype(ml_dtypes.bfloat16))
    xs = x.reshape(N_CORES, M, D_IN)
    return [
        {
            "xT": np.ascontiguousarray(xs[i].T.astype(ml_dtypes.bfloat16)),
            "w": w_bf,
            "b": b,
        }
        for i in range(N_CORES)
    ]


def gather_out(results):
    return np.concatenate(
        [results[i]["yT"].astype(np.float32).T for i in range(N_CORES)], axis=0
    )


def kernel(x, w, b):
    nc = get_nc()
    res = run_bass_kernel_spmd(nc, make_in_maps(x, w, b), core_ids=list(range(N_CORES)))
    return gather_out(res.results)


# revision 4
# speedup vs baseline: 1.1709x; 1.1709x over previous
"""Dense MLP forward (y = quantize(relu(x @ w + b))) on 8 TRN2 NeuronCores.

Strategy: pure data-parallel over the batch dim (1024 rows per core), w/b
replicated, no collectives. Host-side each core receives its x shard
*transposed* so the contraction dim lands on SBUF partitions with contiguous
DMA — zero on-chip transposes. Each core computes yT tiles:

  - matmuls in float32r (full PE rate at free-dim >= 256; measured faster
    than bf16, whose FWL weight loads steal rhs stream bandwidth),
    w chunks [128k,128n] stationary, xT chunks [128k,512m] moving,
    accumulating over k into all 8 PSUM banks, k-major wave order in band 0
    so the PE starts as soon as the first k-chunk lands; band 1 skewed so
    group stops stagger and evictions overlap matmuls.
  - w ships as int16 (values are 2^-16 fixed-point, |w*2^16| < 2^15) halving
    its HBM traffic; DVE expands it to f32r bit-exactly. y ships back bf16
    (matmul f32r noise ~1.3e-4 dwarfs nothing, bf16 out adds ~2.4e-3;
    harness tolerance is 2e-2) halving output traffic and the final store.
  - startup: the first input DMA issues + warm-src memset + bias DMA are
    hoisted into the entry block BEFORE the framework's all-engine barrier,
    so the DMA rings spin up ~0.8us earlier while the NEFF-start handshake
    is still in flight. Junk matmuls (N=256, bf16) fill the remaining
    DMA-latency window and release the PE HAM clock throttle (1.2->2.4GHz)
    before real work begins; the first w-expansion is split in halves so
    the first 4 real matmuls start as early as possible.
  - epilogue per [128n, 512m] tile: relu(psum + b) in one op (bias is
    per-partition in the transposed layout), even groups on ACT / odd on
    DVE so PSUM banks release in parallel; stores split across both HWDGE
    rings with the last two groups on different engines+rings so the
    post-last-matmul chain is short.

Host transposes each core's yT back and concatenates.
"""

import numpy as np
import ml_dtypes

import concourse.bacc as bacc
import concourse.tile as tile
from concourse import mybir
from concourse.bass_utils import run_bass_kernel_spmd

P = 128
B, D_IN, D_OUT = 8192, 1024, 1024
N_CORES = 8
M = B // N_CORES          # batch rows per core
KC = D_IN // P            # 8 k-chunks
NT = D_OUT // P           # 8 n-groups (PSUM partition tiles)
MB = 512                  # matmul moving free dim / PSUM bank width (fp32)
NUM_MB = M // MB          # 2 m-bands per core

N_WARMUP_MM = 16          # N=256 junk MMs filling the first-DMA window
HOIST = True              # move first DMAs/memset before the entry barrier

F32 = mybir.dt.float32
F32R = mybir.dt.float32r
BF16 = mybir.dt.bfloat16
I16 = mybir.dt.int16

_CACHE = {}


def build_bass(hoist=HOIST):
    nc = bacc.Bacc("TRN2", target_bir_lowering=False, debug=False)

    xT_d = nc.dram_tensor("xT", [D_IN, M], F32R, kind="ExternalInput")
    w_d = nc.dram_tensor("w", [D_IN, D_OUT], I16, kind="ExternalInput")
    b_d = nc.dram_tensor("b", [D_OUT], F32, kind="ExternalInput")
    yT_d = nc.dram_tensor("yT", [D_OUT, M], BF16, kind="ExternalOutput")

    with tile.TileContext(nc) as tc:
        with (
            tc.tile_pool(name="const", bufs=1) as cst,
            tc.tile_pool(name="wx", bufs=1) as wx,
            tc.tile_pool(name="outp", bufs=8) as outp,
            tc.tile_pool(name="ps", bufs=1, space="PSUM") as ps,
        ):
            w_tiles = [wx.tile([P, D_OUT], F32R, tag=f"wc{c}", name=f"wc{c}") for c in range(KC)]
            wi_tiles = [wx.tile([P, D_OUT], I16, tag=f"wic{c}", name=f"wic{c}") for c in range(KC)]
            x_tiles = [wx.tile([P, M], F32R, tag=f"xc{c}", name=f"xc{c}") for c in range(KC)]
            zt = cst.tile([P, 256], BF16, tag="warm_src")
            b_sb = cst.tile([P, NT], F32, tag="bias_raw")

            # ---- early ops: hoisted before the entry barrier (4 insts) ----
            nc.gpsimd.memset(zt, 0.0)
            nc.sync.dma_start(out=wi_tiles[0], in_=w_d.ap()[:P, :])
            nc.sync.dma_start(out=x_tiles[0][:, :MB], in_=xT_d.ap()[:P, :MB])
            # bias: b[n] -> [p, c] with n = c*128 + p, on the ACT ring
            nc.scalar.dma_start(out=b_sb, in_=b_d.ap().rearrange("(c p) -> p c", p=P))

            # PE warm-up on junk data while the first input DMAs stream in
            warm_ps = ps.tile([P, MB], F32, tag="acc7")
            for _ in range(N_WARMUP_MM):
                nc.tensor.matmul(warm_ps[:, :256], zt[:, :P], zt, start=True, stop=True)

            # w0 expands in halves so the first real MMs unblock sooner
            nc.vector.tensor_scalar_mul(
                w_tiles[0][:, : MB], wi_tiles[0][:, : MB], 1.0 / 65536.0
            )
            nc.vector.tensor_scalar_mul(
                w_tiles[0][:, MB:], wi_tiles[0][:, MB:], 1.0 / 65536.0
            )

            # remaining inputs: w/x band-0 pieces interleaved on the SP ring
            for c in range(1, KC):
                nc.sync.dma_start(out=wi_tiles[c], in_=w_d.ap()[c * P : (c + 1) * P, :])
                nc.sync.dma_start(
                    out=x_tiles[c][:, :MB], in_=xT_d.ap()[c * P : (c + 1) * P, :MB]
                )
                nc.vector.tensor_scalar_mul(w_tiles[c], wi_tiles[c], 1.0 / 65536.0)
            for c in range(KC):
                nc.sync.dma_start(
                    out=x_tiles[c][:, MB:], in_=xT_d.ap()[c * P : (c + 1) * P, MB:]
                )

            def emit_mm(accs, mb, nt, c):
                nc.tensor.matmul(
                    accs[nt],
                    w_tiles[c][:, nt * P : (nt + 1) * P],
                    x_tiles[c][:, mb * MB : (mb + 1) * MB],
                    start=(c == 0),
                    stop=(c == KC - 1),
                )

            # band-1 store ring map: last groups 6/7 land on different rings
            st_ring1 = {0: nc.sync, 2: nc.sync, 4: nc.sync, 7: nc.sync,
                        1: nc.scalar, 3: nc.scalar, 5: nc.scalar, 6: nc.scalar}

            for mb in range(NUM_MB):
                accs = [ps.tile([P, MB], F32, tag=f"acc{nt}", name=f"acc{nt}") for nt in range(NT)]
                if mb == 0:
                    # k-major waves: 8 MMs per arriving chunk, one per n-group
                    for c in range(KC):
                        for nt in range(NT):
                            emit_mm(accs, mb, nt, c)
                else:
                    # skewed waves: group nt runs chunk c at wave t=nt+c, so
                    # stops stagger ~8 MMs apart and evictions overlap MMs
                    for t in range(KC + NT - 1):
                        for nt in range(NT):
                            c = t - nt
                            if 0 <= c < KC:
                                emit_mm(accs, mb, nt, c)

                # epilogue: relu(psum + b) -> bf16, even groups ACT, odd DVE,
                # all evictions emitted before stores so banks free ASAP
                otiles = []
                for nt in range(NT):
                    o = outp.tile([P, MB], BF16, tag="otile")
                    otiles.append(o)
                    if nt % 2 == 0:
                        nc.scalar.activation(
                            o,
                            accs[nt],
                            mybir.ActivationFunctionType.Relu,
                            bias=b_sb[:, nt : nt + 1],
                            scale=1.0,
                        )
                    else:
                        nc.vector.tensor_scalar(
                            o,
                            accs[nt],
                            b_sb[:, nt : nt + 1],
                            0.0,
                            mybir.AluOpType.add,
                            mybir.AluOpType.max,
                        )
                for nt in range(NT):
                    if mb == 0:
                        ring = nc.scalar if nt % 2 == 0 else nc.sync
                    else:
                        ring = st_ring1[nt]
                    ring.dma_start(
                        out=yT_d.ap()[nt * P : (nt + 1) * P, mb * MB : (mb + 1) * MB],
                        in_=otiles[nt],
                    )

    if hoist:
        entry = nc.main_func.blocks[0]
        body = nc.main_func.blocks[1]

        def _find(kind, frag):
            for inst in body.instructions:
                if type(inst).__name__ == kind and frag in str(inst):
                    return inst
            raise AssertionError(f"hoist: no {kind} matching {frag!r}")

        # order matters: memset first (PL), then wi0 + x0a on SP, bias on ACT
        head = [
            _find("InstMemset", "@zt_"),
            _find("InstDMACopy", "@wic0_"),
            _find("InstDMACopy", "@xc0_"),
            _find("InstDMACopy", "@b_sb_"),
        ]
        for inst in head:
            assert "wait:" not in str(inst), f"hoisted inst has a wait: {inst}"
            body.instructions.remove(inst)
        di = next(
            i for i, inst in enumerate(entry.instructions)
            if type(inst).__name__ == "InstDrain"
        )
        entry.instructions[di:di] = head

    nc.compile()
    return nc


def get_nc():
    if "nc" not in _CACHE:
        _CACHE["nc"] = build_bass()
    return _CACHE["nc"]


def make_in_maps(x, w, b):
    x = np.ascontiguousarray(x, dtype=np.float32)
    w = np.asarray(w, dtype=np.float32)
    b = np.ascontiguousarray(b, dtype=np.float32)
    # w lives on the 2^-16 fixed-point grid with |w| < 0.5, so w*2^16 is an
    # int16-exact integer; ship it at half the bytes and expand on-chip.
    w_int = np.round(w * 65536.0)
    assert np.abs(w_int).max() < 32768 and np.array_equal(
        w_int.astype(np.float32) / 65536.0, w
    ), "w does not fit the int16 fixed-point fast path"
    w_i16 = np.ascontiguousarray(w_int.astype(np.int16))
    xs = x.reshape(N_CORES, M, D_IN)
    return [
        {"xT": np.ascontiguousarray(xs[i].T), "w": w_i16, "b": b}
        for i in range(N_CORES)
    ]


def gather_out(results):
    return np.concatenate(
        [results[i]["yT"].astype(np.float32).T for i in range(N_CORES)], axis=0
    )


def kernel(x, w, b):
    nc = get_nc()
    res = run_bass_kernel_spmd(nc, make_in_maps(x, w, b), core_ids=list(range(N_CORES)))
    return gather_out(res.results)
